# revision 1
# baseline (speedup 1.0000x reference)
"""MoE top-2 routing kernel for 8 Trainium2 NeuronCores.

Problem (hardcoded shapes): x [64,8,2048] f32, gate_w [2048,8] f32,
w1/w3 [8,2048,4096] f32, w2 [8,4096,2048] f32, top_k=2.

Strategy (expert parallelism):
  - Host computes the gate (512x8 logits, top-2, softmax) exactly as the
    reference does -- this is ~17 MFLOP, negligible.
  - Tokens are dispatched per expert (gathered + padded to capacity C),
    one expert per NeuronCore.  Each core runs the SwiGLU FFN for its
    expert over its C token slots:
        outT = w2^T @ (silu(w1^T @ xT) * (w3^T @ xT))
    with all matmuls laid out [K, M]/[K, N] so no on-device transposes
    are needed (tokens are the moving free dim).
  - The combine weights are folded into the host-side scatter-add of the
    per-expert outputs back into the [512, 2048] output.

MM_DTYPE selects the matmul precision:
  "f32r": full fp32 inputs, tf32-class PE mode (full rate at free dim
          >= 256); measured rel err vs fp32 reference ~2.6e-4.
  "bf16": weights/activations cast to bf16 on host (halves the HBM
          traffic, which is the roofline); rel err ~5e-3.
"""

import numpy as np

B, S, D, F, E = 64, 8, 2048, 4096, 8
T = B * S  # 512 tokens
P = 128
KD = D // P   # 16 k-tiles, D contraction
KF = F // P   # 32 k-tiles, F contraction
MF = F // P   # 32 m-tiles, stage 1
MD = D // P   # 16 m-tiles, stage 2
G1 = 4        # stage-1 m-tiles per group (4 gate + 4 up PSUM tiles = 8 banks)
G2 = 8        # stage-2 m-tiles per group (8 PSUM tiles = 8 banks)

MM_DTYPE = "bf16"   # "f32r" | "bf16"
W_BUFS = 20

_cache = {}
last_results = None  # BassKernelResults of the most recent device run


def _np_dt(mode):
    if mode == "bf16":
        import ml_dtypes
        return np.dtype(ml_dtypes.bfloat16)
    return np.dtype(np.float32)


def _build(C, mode, w_bufs=None):
    import concourse.mybir as mybir
    import concourse.tile as tile
    from concourse import bacc

    if w_bufs is None:
        w_bufs = 16 if mode == "bf16" else 10
    nc = bacc.Bacc(None, target_bir_lowering=False)
    f32 = mybir.dt.float32
    mmdt = {"f32r": mybir.dt.float32r, "bf16": mybir.dt.bfloat16}[mode]

    NG1 = MF // G1          # 8 stage-1 groups (512 cols each)
    NG2 = MD // G2          # 2 stage-2 groups (1024 cols each)
    # weights packed on host so each dma_start moves one [128, 4KB] block:
    #   w13 [NG1, KD//2, 128, kk=2, w=2, G1*128]
    #   w2p [NG2, KF//2, 128, kk=2, G2*128]
    w13 = nc.declare_dram_parameter("w13", [NG1, KD // 2, P, 2, 2, G1 * P],
                                    mmdt, isOutput=False)
    w2p = nc.declare_dram_parameter("w2p", [NG2, KF // 2, P, 2, G2 * P],
                                    mmdt, isOutput=False)
    xT = nc.declare_dram_parameter("xT", [P, KD, C], mmdt, isOutput=False)
    outT = nc.declare_dram_parameter("outT", [NG2, P, G2, C], f32, isOutput=True)

    with tile.TileContext(nc) as tc:
        with (
            tc.tile_pool(name="xpool", bufs=1) as xpool,
            tc.tile_pool(name="hpool", bufs=1) as hpool,
            tc.tile_pool(name="wpool", bufs=w_bufs) as wpool,
            tc.tile_pool(name="wpool2", bufs=11) as wpool2,
            tc.tile_pool(name="psum", bufs=8, space="PSUM") as psum,
            tc.tile_pool(name="spool", bufs=G1 * 2) as spool,
            tc.tile_pool(name="opool", bufs=2) as opool,
        ):
            xt = xpool.tile([P, KD, C], mmdt)
            nc.sync.dma_start(out=xt[:, 0:4, :], in_=xT[:, 0:4, :])
            nc.scalar.dma_start(out=xt[:, 4:, :], in_=xT[:, 4:, :])
            ht = hpool.tile([P, KF, C], mmdt)

            dma_eng = [nc.sync, nc.scalar]
            ndma = 0

            warm = xpool.tile([P, 256], mmdt, name="warm")
            nc.vector.memset(warm[:], 0.0)
            ps_w = psum.tile([P, C], f32, tag="ps", name="ps_warm")
            for i in range(40):
                nc.tensor.matmul(ps_w[:], warm[:, :P], warm[:, :C],
                                 start=True, stop=True)

            # stage 1: hT[f, t] = silu(w1^T xT) * (w3^T xT), F-major groups
            for g in range(NG1):
                ps_g = [psum.tile([P, C], f32, tag="ps", name=f"ps_g{g}_{m}")
                        for m in range(G1)]
                ps_u = [psum.tile([P, C], f32, tag="ps", name=f"ps_u{g}_{m}")
                        for m in range(G1)]
                for kp in range(KD // 2):
                    wt = wpool.tile([P, 2, 2, G1 * P], mmdt, tag="w")
                    dma_eng[ndma % 2].dma_start(out=wt[:], in_=w13[g, kp])
                    ndma += 1
                    for kk in range(2):
                        k = kp * 2 + kk
                        st, sp = (k == 0), (k == KD - 1)
                        for m in range(G1):
                            nc.tensor.matmul(ps_g[m][:], wt[:, kk, 0, m * P:(m + 1) * P],
                                             xt[:, k, :], start=st, stop=sp)
                            nc.tensor.matmul(ps_u[m][:], wt[:, kk, 1, m * P:(m + 1) * P],
                                             xt[:, k, :], start=st, stop=sp)
                for m in range(G1):
                    sig = spool.tile([P, C], f32, tag="sig")
                    nc.scalar.activation(sig[:], ps_g[m][:],
                                         mybir.ActivationFunctionType.Silu)
                    nc.vector.tensor_tensor(out=ht[:, g * G1 + m, :], in0=sig[:],
                                            in1=ps_u[m][:], op=mybir.AluOpType.mult)

            # stage 2: outT[d, t] = w2^T @ hT
            for g in range(NG2):
                ps_o = [psum.tile([P, C], f32, tag="ps", name=f"ps_o{g}_{m}")
                        for m in range(G2)]
                for kp in range(KF // 2):
                    wt = wpool2.tile([P, 2, G2 * P], mmdt, tag="w2")
                    dma_eng[ndma % 2].dma_start(out=wt[:], in_=w2p[g, kp])
                    ndma += 1
                    for kk in range(2):
                        k = kp * 2 + kk
                        st, sp = (k == 0), (k == KF - 1)
                        for m in range(G2):
                            nc.tensor.matmul(ps_o[m][:], wt[:, kk, m * P:(m + 1) * P],
                                             ht[:, k, :], start=st, stop=sp)
                obuf = opool.tile([P, G2, C], f32, tag="o", name=f"ob{g}")
                for m in range(G2):
                    nc.vector.tensor_copy(out=obuf[:, m, :], in_=ps_o[m][:])
                nc.sync.dma_start(out=outT[g], in_=obuf[:])

    nc.compile()
    return nc


def _route(x2d, gate_w, top_k):
    """Replicates the reference gate on host: returns (sel [T,k], cw [T,k])."""
    logits = x2d @ gate_w                       # [T, E] fp32
    sel = np.argsort(-logits, axis=-1, kind="stable")[:, :top_k]
    vals = np.take_along_axis(logits, sel, axis=-1)
    m = vals.max(axis=-1, keepdims=True)
    ex = np.exp(vals - m)
    cw = ex / ex.sum(axis=-1, keepdims=True)
    return sel, cw


def kernel(x, gate_w, w1, w3, w2, top_k):
    from concourse.bass_utils import run_bass_kernel_spmd

    x = np.asarray(x, np.float32)
    gate_w = np.asarray(gate_w, np.float32)
    w1 = np.asarray(w1, np.float32)
    w3 = np.asarray(w3, np.float32)
    w2 = np.asarray(w2, np.float32)
    k = int(top_k)

    x2d = x.reshape(T, D)
    sel, cw = _route(x2d, gate_w, k)

    # token lists per expert
    idx = [np.where((sel == e).any(axis=1))[0] for e in range(E)]
    wgt = []
    for e in range(E):
        m = sel[idx[e]] == e
        wgt.append(cw[idx[e]][m].astype(np.float32))
    counts = np.array([len(i) for i in idx])
    maxc = int(counts.max())
    if MM_DTYPE == "f32r":
        C = max(256, -(-maxc // 64) * 64)
    else:
        C = max(192, -(-maxc // 64) * 64)
    n_chunks = 1
    if C > 512:  # capacity overflow: run multiple passes of 512
        C = 512
        n_chunks = -(-maxc // C)

    key = (C, MM_DTYPE)
    if key not in _cache:
        _cache[key] = _build(C, MM_DTYPE)
    nc = _cache[key]

    ndt = _np_dt(MM_DTYPE)
    NG1, NG2 = MF // G1, MD // G2
    wpacked = []
    for e in range(E):
        # w13 [NG1, KD//2, P, kk, w, G1*P]: line = one 4KB block per partition
        w1r = w1[e].astype(ndt).reshape(KD // 2, 2, P, NG1, G1 * P)
        w3r = w3[e].astype(ndt).reshape(KD // 2, 2, P, NG1, G1 * P)
        w13 = np.ascontiguousarray(
            np.stack([w1r, w3r], axis=4).transpose(3, 0, 2, 1, 4, 5))
        # w2p [NG2, KF//2, P, kk, G2*P]
        w2r = w2[e].astype(ndt).reshape(KF // 2, 2, P, NG2, G2 * P)
        w2pk = np.ascontiguousarray(w2r.transpose(3, 0, 2, 1, 4))
        wpacked.append((w13, w2pk))

    out = np.zeros((T, D), np.float32)
    for chunk in range(n_chunks):
        in_maps = []
        for e in range(E):
            ide = idx[e][chunk * C:(chunk + 1) * C]
            xTe = np.zeros((D, C), ndt)
            xTe[:, :len(ide)] = x2d[ide].T.astype(ndt)
            in_maps.append({
                "xT": np.ascontiguousarray(
                    xTe.reshape(KD, P, C).transpose(1, 0, 2)),
                "w13": wpacked[e][0],
                "w2p": wpacked[e][1],
            })
        res = run_bass_kernel_spmd(nc, in_maps, core_ids=list(range(E)))
        global last_results
        last_results = res
        for e in range(E):
            ide = idx[e][chunk * C:(chunk + 1) * C]
            if len(ide) == 0:
                continue
            we = wgt[e][chunk * C:(chunk + 1) * C]
            # outT [NG2, P, G2, C] -> [D, C] with d = g*G2*P + m*P + p
            oTe = res.results[e]["outT"].transpose(0, 2, 1, 3).reshape(D, C)
            # token indices are unique within one expert's list
            out[ide] += we[:, None] * oTe[:, :len(ide)].T

    return out.reshape(B, S, D)



# revision 4
# speedup vs baseline: 1.2254x; 1.2254x over previous
"""MoE top-2 routing kernel for 8 Trainium2 NeuronCores.

Problem (hardcoded shapes): x [64,8,2048] f32, gate_w [2048,8] f32,
w1/w3 [8,2048,4096] f32, w2 [8,4096,2048] f32, top_k=2.

Strategy (expert parallelism):
  - Host computes the gate (512x8 logits, top-2, softmax) exactly as the
    reference does -- ~17 MFLOP, negligible.
  - Tokens are dispatched per expert (gathered + padded to capacity C),
    one expert per NeuronCore.  Each core runs the SwiGLU FFN for its
    expert over its C token slots:
        outT = w2^T @ (silu(w1^T @ xT) * (w3^T @ xT))
    with all matmuls laid out [K, M]/[K, N] so no on-device transposes
    are needed (tokens are the moving free dim).
  - Combine weights are folded into the host-side scatter-add.

Precision plan (rel-err budget 2e-2, measured against the fp32 ref):
  - activations bf16, w1 bf16          -> base err ~4e-3
  - w2 fp8 e3m4, per-out-column scale  -> +~1.3e-2 (scale applied on
    host to the output rows: free)
  - w3 fp8 e3m4 for W3_NFP8/16 of its column groups, per-column scale
    folded into w2's rows on host (exact, free on device)
  Mixed-dtype matmul (fp8 weights x bf16 activations) is legal on the
  PE; only fp32 must match on both sides.

Perf plan (per-core roofline: PE ~1536 LDW/MM pairs; HBM 358 GB/s):
  - C = tokens padded to 32 (>=128) instead of 64/192: fewer MM columns
  - stage-1 groups of G1=2 m-tiles (4 PSUM banks) and stage-2 groups of
    G2=4 (4 banks) so consecutive groups ping-pong PSUM banks and the
    PE never stalls on drains.
  - deep weight pools so the DMA queues stay >=4 groups ahead; weights
    packed so every dma_start moves 256-512KB with 2-4KB partition
    lines.
"""

import numpy as np

B, S, D, F, E = 64, 8, 2048, 4096, 8
T = B * S  # 512 tokens
P = 128
KD = D // P   # 16 k-tiles, D contraction
KF = F // P   # 32 k-tiles, F contraction
MF = F // P   # 32 m-tiles, stage 1
MD = D // P   # 16 m-tiles, stage 2
G1 = 2        # stage-1 m-tiles per group (2 gate + 2 up = 4 PSUM banks)
G2 = 4        # stage-2 m-tiles per group (4 PSUM banks)
NG1 = MF // G1   # 16 stage-1 groups
NG2 = MD // G2   # 4 stage-2 groups

W3_NFP8 = 16    # how many of the NG1 stage-1 groups keep w3 in fp8
W2_FP8 = True
F8MAX = 14.0    # per-column scale target (e3m4 max normal = 15.5)

_cache = {}
last_results = None  # BassKernelResults of the most recent device run


def _np_bf16():
    import ml_dtypes
    return np.dtype(ml_dtypes.bfloat16)


def _np_f8():
    import ml_dtypes
    return np.dtype(ml_dtypes.float8_e3m4)


def _build(C, n3f, w2fp8):
    import concourse.mybir as mybir
    import concourse.tile as tile
    from concourse import bacc

    nc = bacc.Bacc(None, target_bir_lowering=False)
    f32 = mybir.dt.float32
    bf16 = mybir.dt.bfloat16
    f8 = mybir.dt.float8e3
    dt2 = f8 if w2fp8 else bf16
    n3b = NG1 - n3f

    # weights packed on host; each dma_start moves one (g, kp) block with
    # a contiguous 2-4KB line per partition:
    #   w1p [NG1, 2, P, KD//2, G1, P]   bf16 (4KB lines, 512KB/DMA)
    #   w3f [n3f, 2, P, KD//2, G1, P]   f8e3 (2KB lines, 256KB/DMA)
    #   w3b [n3b, 2, P, KD//2, G1, P]   bf16
    #   w2p [NG2, 4, P, KF//4, G2, P]   f8e3 (4KB lines, 512KB/DMA)
    w1p = nc.declare_dram_parameter("w1p", [NG1, 2, P, KD // 2, G1, P],
                                    bf16, isOutput=False)
    if n3f:
        w3f = nc.declare_dram_parameter("w3f", [n3f, 2, P, KD // 2, G1, P],
                                        f8, isOutput=False)
    if n3b:
        w3b = nc.declare_dram_parameter("w3b", [n3b, 2, P, KD // 2, G1, P],
                                        bf16, isOutput=False)
    w2p = nc.declare_dram_parameter("w2p", [NG2, 4, P, KF // 4, G2, P],
                                    dt2, isOutput=False)
    xT = nc.declare_dram_parameter("xT", [P, KD, C], bf16, isOutput=False)
    outT = nc.declare_dram_parameter("outT", [NG2, P, G2, C], f32,
                                     isOutput=True)

    with tile.TileContext(nc) as tc:
        with (
            tc.tile_pool(name="xpool", bufs=1) as xpool,
            tc.tile_pool(name="hpool", bufs=1) as hpool,
            tc.tile_pool(name="w1pool", bufs=10) as w1pool,
            tc.tile_pool(name="w3pool", bufs=10) as w3pool,
            tc.tile_pool(name="w2pool", bufs=8) as w2pool,
            tc.tile_pool(name="psum", bufs=8, space="PSUM") as psum,
            tc.tile_pool(name="spool", bufs=4) as spool,
            tc.tile_pool(name="opool", bufs=2) as opool,
        ):
            xt = xpool.tile([P, KD, C], bf16)
            nc.sync.dma_start(out=xt[:, 0:8, :], in_=xT[:, 0:8, :])
            nc.scalar.dma_start(out=xt[:, 8:, :], in_=xT[:, 8:, :])
            ht = hpool.tile([P, KF, C], bf16)

            dma_eng = [nc.sync, nc.scalar]
            ndma = 0

            # keep the PE busy from t=0 so HAM is un-throttled by the
            # time the first real matmul issues (~3.4us of activity)
            warm = xpool.tile([P, 256], bf16, name="warm")
            nc.vector.memset(warm[:], 0.0)
            ps_w = psum.tile([P, 256], f32, tag="ps", name="ps_warm")
            for i in range(22):
                nc.tensor.matmul(ps_w[:], warm[:, :P], warm[:],
                                 start=True, stop=True)

            # stage 1: hT[f, t] = silu(w1^T xT) * (w3^T xT)
            for g in range(NG1):
                w3_is_f8 = g < n3f
                dt3 = f8 if w3_is_f8 else bf16
                w3src = (w3f[g] if w3_is_f8 else w3b[g - n3f])
                ps_g = [psum.tile([P, C], f32, tag="ps", name=f"ps_g{g}_{m}")
                        for m in range(G1)]
                ps_u = [psum.tile([P, C], f32, tag="ps", name=f"ps_u{g}_{m}")
                        for m in range(G1)]
                for kp in range(2):
                    wt1 = w1pool.tile([P, KD // 2, G1, P], bf16, tag="w1")
                    dma_eng[ndma % 2].dma_start(out=wt1[:], in_=w1p[g, kp])
                    ndma += 1
                    wt3 = w3pool.tile([P, KD // 2, G1, P], dt3, tag="w3")
                    dma_eng[ndma % 2].dma_start(out=wt3[:], in_=w3src[kp])
                    ndma += 1
                    for kk in range(KD // 2):
                        k = kp * (KD // 2) + kk
                        st, sp = (k == 0), (k == KD - 1)
                        for m in range(G1):
                            nc.tensor.matmul(ps_g[m][:], wt1[:, kk, m, :],
                                             xt[:, k, :], start=st, stop=sp)
                            nc.tensor.matmul(ps_u[m][:], wt3[:, kk, m, :],
                                             xt[:, k, :], start=st, stop=sp)
                for m in range(G1):
                    sig = spool.tile([P, C], f32, tag="sig")
                    nc.scalar.activation(sig[:], ps_g[m][:],
                                         mybir.ActivationFunctionType.Silu)
                    nc.vector.tensor_tensor(out=ht[:, g * G1 + m, :],
                                            in0=sig[:], in1=ps_u[m][:],
                                            op=mybir.AluOpType.mult)

            # stage 2: outT[d, t] = w2^T @ hT
            for g in range(NG2):
                ps_o = [psum.tile([P, C], f32, tag="ps", name=f"ps_o{g}_{m}")
                        for m in range(G2)]
                for kp in range(4):
                    wt2 = w2pool.tile([P, KF // 4, G2, P], dt2, tag="w2")
                    dma_eng[ndma % 2].dma_start(out=wt2[:], in_=w2p[g, kp])
                    ndma += 1
                    for kk in range(KF // 4):
                        k = kp * (KF // 4) + kk
                        st, sp = (k == 0), (k == KF - 1)
                        for m in range(G2):
                            nc.tensor.matmul(ps_o[m][:], wt2[:, kk, m, :],
                                             ht[:, k, :], start=st, stop=sp)
                obuf = opool.tile([P, G2, C], f32, tag="o", name=f"ob{g}")
                for m in range(G2):
                    nc.vector.tensor_copy(out=obuf[:, m, :], in_=ps_o[m][:])
                dma_eng[ndma % 2].dma_start(out=outT[g], in_=obuf[:])
                ndma += 1

    nc.compile()
    return nc


def _route(x2d, gate_w, top_k):
    """Replicates the reference gate on host: returns (sel [T,k], cw [T,k])."""
    logits = x2d @ gate_w                       # [T, E] fp32
    sel = np.argsort(-logits, axis=-1, kind="stable")[:, :top_k]
    vals = np.take_along_axis(logits, sel, axis=-1)
    m = vals.max(axis=-1, keepdims=True)
    ex = np.exp(vals - m)
    cw = ex / ex.sum(axis=-1, keepdims=True)
    return sel, cw


def _pack_s1(w, dt):
    # [D, ncols] -> [ng, 2, P, KD//2, G1, P]: block (g, kp) holds k-tiles
    # kp*8..kp*8+7 x m-tiles g*G1..g*G1+G1-1, one 2-4KB line per partition
    ng = w.shape[1] // (G1 * P)
    r = w.astype(dt).reshape(2, KD // 2, P, ng, G1, P)
    return np.ascontiguousarray(r.transpose(3, 0, 2, 1, 4, 5))


def _pack_s2(w, dt):
    # [F, D] -> [NG2, 4, P, KF//4, G2, P]
    r = w.astype(dt).reshape(4, KF // 4, P, NG2, G2, P)
    return np.ascontiguousarray(r.transpose(3, 0, 2, 1, 4, 5))


def kernel(x, gate_w, w1, w3, w2, top_k):
    from concourse.bass_utils import run_bass_kernel_spmd

    x = np.asarray(x, np.float32)
    gate_w = np.asarray(gate_w, np.float32)
    w1 = np.asarray(w1, np.float32)
    w3 = np.asarray(w3, np.float32)
    w2 = np.asarray(w2, np.float32)
    k = int(top_k)

    x2d = x.reshape(T, D)
    sel, cw = _route(x2d, gate_w, k)

    # token lists per expert
    idx = [np.where((sel == e).any(axis=1))[0] for e in range(E)]
    wgt = []
    for e in range(E):
        m = sel[idx[e]] == e
        wgt.append(cw[idx[e]][m].astype(np.float32))
    counts = np.array([len(i) for i in idx])
    maxc = int(counts.max())
    C = max(128, -(-maxc // 32) * 32)
    n_chunks = 1
    if C > 512:  # capacity overflow: run multiple passes of 512
        C = 512
        n_chunks = -(-maxc // C)

    key = (C, W3_NFP8, W2_FP8)
    if key not in _cache:
        _cache[key] = _build(C, W3_NFP8, W2_FP8)
    nc = _cache[key]

    bf16 = _np_bf16()
    f8 = _np_f8()
    dt2 = f8 if W2_FP8 else bf16

    wpacked = []
    for e in range(E):
        # per-column scale for the fp8 part of w3; folded into w2's rows
        s3 = np.ones(F, np.float32)
        w3e = w3[e]
        if W3_NFP8:
            nf = W3_NFP8 * G1 * P  # first nf columns of F are fp8 groups
            s3[:nf] = np.abs(w3e[:, :nf]).max(axis=0) / F8MAX
            w3e = w3e / s3[None, :]
        # w2 rows absorb s3; its own per-column scale s2 is applied on host
        w2pre = w2[e] * s3[:, None]
        if W2_FP8:
            s2 = np.abs(w2pre).max(axis=0) / F8MAX
            w2pre = w2pre / s2[None, :]
        else:
            s2 = np.ones(D, np.float32)

        nf = W3_NFP8 * G1 * P
        maps = {"w1p": _pack_s1(w1[e], bf16), "w2p": _pack_s2(w2pre, dt2)}
        if W3_NFP8:
            maps["w3f"] = _pack_s1(w3e[:, :nf], f8)
        if W3_NFP8 < NG1:
            maps["w3b"] = _pack_s1(w3e[:, nf:], bf16)
        wpacked.append((maps, s2))

    out = np.zeros((T, D), np.float32)
    for chunk in range(n_chunks):
        in_maps = []
        for e in range(E):
            ide = idx[e][chunk * C:(chunk + 1) * C]
            xTe = np.zeros((D, C), bf16)
            xTe[:, :len(ide)] = x2d[ide].T.astype(bf16)
            m = dict(wpacked[e][0])
            m["xT"] = np.ascontiguousarray(
                xTe.reshape(KD, P, C).transpose(1, 0, 2))
            in_maps.append(m)
        res = run_bass_kernel_spmd(nc, in_maps, core_ids=list(range(E)))
        global last_results
        last_results = res
        for e in range(E):
            ide = idx[e][chunk * C:(chunk + 1) * C]
            if len(ide) == 0:
                continue
            we = wgt[e][chunk * C:(chunk + 1) * C]
            s2 = wpacked[e][1]
            # outT [NG2, P, G2, C] -> [D, C] with d = g*G2*P + m*P + p
            oTe = res.results[e]["outT"].transpose(0, 2, 1, 3).reshape(D, C)
            out[ide] += we[:, None] * (oTe[:, :len(ide)].T * s2[None, :])

    return out.reshape(B, S, D)


# revision 5
# speedup vs baseline: 1.3711x; 1.1189x over previous
"""MoE top-2 routing kernel for 8 Trainium2 NeuronCores.

Problem (hardcoded shapes): x [64,8,2048] f32, gate_w [2048,8] f32,
w1/w3 [8,2048,4096] f32, w2 [8,4096,2048] f32, top_k=2.

Strategy (expert parallelism):
  - Host computes the gate (512x8 logits, top-2, softmax) exactly as the
    reference does -- ~17 MFLOP, negligible.
  - Tokens are dispatched per expert (gathered + padded to capacity C),
    one expert per NeuronCore.  Each core runs the SwiGLU FFN for its
    expert over its C token slots:
        outT = w2^T @ (silu(w1^T @ xT) * (w3^T @ xT))
    with all matmuls laid out [K, M]/[K, N] so no on-device transposes
    are needed (tokens are the moving free dim).
  - Combine weights are folded into the host-side scatter-add.

Precision plan (rel-err budget 2e-2, measured against the fp32 ref):
  - activations bf16, w1 bf16
  - w3, w2 in fp8 e3m4 with per-output-column scales; w3's scale is
    folded into w2's rows on host (exact), w2's scale into the host
    combine (exact).  Mixed-dtype matmul (fp8 weights x bf16
    activations) is legal on the PE.  Measured rel err 1.90e-2 on HW,
    bit-identical to the numpy prediction (deterministic inputs).

Perf notes (per-core roofline: PE 1536 LDW/MM pairs ~ C/2.4ns each;
HBM ~358 GB/s; stage-1 weight demand slightly exceeds HBM supply so
stage 1 is DMA-paced, stage 2 is PE-paced):
  - C = tokens padded to 16 (>=128)
  - one 0.5-1MB dma_start per (group, tensor) with 4-8KB partition
    lines; queues byte-balanced by alternating groups; outT stores on
    the gpsimd (SWDGE) queue so they never head-of-line-block weights.
  - stage-1 groups of G1=2 m-tiles (4 PSUM banks) and stage-2 groups
    of G2=4 (4 banks): consecutive groups ping-pong PSUM banks.
"""

import numpy as np

B, S, D, F, E = 64, 8, 2048, 4096, 8
T = B * S  # 512 tokens
P = 128
KD = D // P   # 16 k-tiles, D contraction
KF = F // P   # 32 k-tiles, F contraction
MF = F // P   # 32 m-tiles, stage 1
MD = D // P   # 16 m-tiles, stage 2
G1 = 2        # stage-1 m-tiles per group (2 gate + 2 up = 4 PSUM banks)
G2 = 4        # stage-2 m-tiles per group (4 PSUM banks)
NG1 = MF // G1   # 16 stage-1 groups
NG2 = MD // G2   # 4 stage-2 groups

W3_NFP8 = 16    # how many of the NG1 stage-1 groups keep w3 in fp8
W2_FP8 = True
F8MAX = 14.0    # per-column scale target (e3m4 max normal = 15.5)

_cache = {}
last_results = None  # BassKernelResults of the most recent device run


def _np_bf16():
    import ml_dtypes
    return np.dtype(ml_dtypes.bfloat16)


def _np_f8():
    import ml_dtypes
    return np.dtype(ml_dtypes.float8_e3m4)


def _build(C, n3f, w2fp8):
    import concourse.mybir as mybir
    import concourse.tile as tile
    from concourse import bacc

    nc = bacc.Bacc(None, target_bir_lowering=False)
    f32 = mybir.dt.float32
    bf16 = mybir.dt.bfloat16
    f8 = mybir.dt.float8e3
    dt2 = f8 if w2fp8 else bf16
    n3b = NG1 - n3f

    # weights packed on host; each dma_start moves one whole group block
    # with a contiguous 4-8KB line per partition:
    #   w1p [NG1, P, KD, G1, P]     bf16 (8KB lines, 1MB/DMA)
    #   w3f [n3f, P, KD, G1, P]     f8e3 (4KB lines, 0.5MB/DMA)
    #   w3b [n3b, P, KD, G1, P]     bf16
    #   w2p [NG2, 2, P, KF//2, G2, P]  f8e3 (8KB lines, 1MB/DMA)
    w1p = nc.declare_dram_parameter("w1p", [NG1, P, KD, G1, P],
                                    bf16, isOutput=False)
    if n3f:
        w3f = nc.declare_dram_parameter("w3f", [n3f, P, KD, G1, P],
                                        f8, isOutput=False)
    if n3b:
        w3b = nc.declare_dram_parameter("w3b", [n3b, P, KD, G1, P],
                                        bf16, isOutput=False)
    w2p = nc.declare_dram_parameter("w2p", [NG2, 2, P, KF // 2, G2, P],
                                    dt2, isOutput=False)
    xT = nc.declare_dram_parameter("xT", [P, KD, C], bf16, isOutput=False)
    outT = nc.declare_dram_parameter("outT", [NG2, P, G2, C], f32,
                                     isOutput=True)

    with tile.TileContext(nc) as tc:
        with (
            tc.tile_pool(name="xpool", bufs=1) as xpool,
            tc.tile_pool(name="hpool", bufs=1) as hpool,
            tc.tile_pool(name="w1pool", bufs=4) as w1pool,
            tc.tile_pool(name="w3pool", bufs=4) as w3pool,
            tc.tile_pool(name="w2pool", bufs=4) as w2pool,
            tc.tile_pool(name="psum", bufs=8, space="PSUM") as psum,
            tc.tile_pool(name="spool", bufs=4) as spool,
            tc.tile_pool(name="opool", bufs=2) as opool,
        ):
            xt = xpool.tile([P, KD, C], bf16)
            nc.sync.dma_start(out=xt[:, 0:8, :], in_=xT[:, 0:8, :])
            nc.scalar.dma_start(out=xt[:, 8:, :], in_=xT[:, 8:, :])
            ht = hpool.tile([P, KF, C], bf16)

            dma_eng = [nc.sync, nc.scalar]

            # keep the PE busy from t=0 so HAM is un-throttled by the
            # time the first real matmul issues (~3.4us of activity)
            warm = xpool.tile([P, 256], bf16, name="warm")
            nc.vector.memset(warm[:], 0.0)
            ps_w = psum.tile([P, 256], f32, tag="ps", name="ps_warm")
            for i in range(24):
                nc.tensor.matmul(ps_w[:], warm[:, :P], warm[:],
                                 start=True, stop=True)

            # stage 1: hT[f, t] = silu(w1^T xT) * (w3^T xT)
            for g in range(NG1):
                w3_is_f8 = g < n3f
                dt3 = f8 if w3_is_f8 else bf16
                w3src = (w3f[g] if w3_is_f8 else w3b[g - n3f])
                wt1 = w1pool.tile([P, KD, G1, P], bf16, tag="w1")
                dma_eng[g % 2].dma_start(out=wt1[:], in_=w1p[g])
                wt3 = w3pool.tile([P, KD, G1, P], dt3, tag="w3")
                dma_eng[(g + 1) % 2].dma_start(out=wt3[:], in_=w3src)
                ps_g = [psum.tile([P, C], f32, tag="ps", name=f"ps_g{g}_{m}")
                        for m in range(G1)]
                ps_u = [psum.tile([P, C], f32, tag="ps", name=f"ps_u{g}_{m}")
                        for m in range(G1)]
                for k in range(KD):
                    st, sp = (k == 0), (k == KD - 1)
                    for m in range(G1):
                        nc.tensor.matmul(ps_g[m][:], wt1[:, k, m, :],
                                         xt[:, k, :], start=st, stop=sp)
                        nc.tensor.matmul(ps_u[m][:], wt3[:, k, m, :],
                                         xt[:, k, :], start=st, stop=sp)
                for m in range(G1):
                    sig = spool.tile([P, C], f32, tag="sig")
                    nc.scalar.activation(sig[:], ps_g[m][:],
                                         mybir.ActivationFunctionType.Silu)
                    nc.vector.tensor_tensor(out=ht[:, g * G1 + m, :],
                                            in0=sig[:], in1=ps_u[m][:],
                                            op=mybir.AluOpType.mult)

            # stage 2: outT[d, t] = w2^T @ hT
            for g in range(NG2):
                ps_o = [psum.tile([P, C], f32, tag="ps", name=f"ps_o{g}_{m}")
                        for m in range(G2)]
                for kp in range(2):
                    wt2 = w2pool.tile([P, KF // 2, G2, P], dt2, tag="w2")
                    dma_eng[(g + kp) % 2].dma_start(out=wt2[:], in_=w2p[g, kp])
                    for kk in range(KF // 2):
                        k = kp * (KF // 2) + kk
                        st, sp = (k == 0), (k == KF - 1)
                        for m in range(G2):
                            nc.tensor.matmul(ps_o[m][:], wt2[:, kk, m, :],
                                             ht[:, k, :], start=st, stop=sp)
                obuf = opool.tile([P, G2, C], f32, tag="o", name=f"ob{g}")
                for m in range(G2):
                    nc.vector.tensor_copy(out=obuf[:, m, :], in_=ps_o[m][:])
                nc.gpsimd.dma_start(out=outT[g], in_=obuf[:])

    nc.compile()
    return nc


def _route(x2d, gate_w, top_k):
    """Replicates the reference gate on host: returns (sel [T,k], cw [T,k])."""
    logits = x2d @ gate_w                       # [T, E] fp32
    sel = np.argsort(-logits, axis=-1, kind="stable")[:, :top_k]
    vals = np.take_along_axis(logits, sel, axis=-1)
    m = vals.max(axis=-1, keepdims=True)
    ex = np.exp(vals - m)
    cw = ex / ex.sum(axis=-1, keepdims=True)
    return sel, cw


def _pack_s1(w, dt):
    # [D, ncols] -> [ng, P, KD, G1, P]: block g holds all KD k-tiles of
    # m-tiles g*G1..g*G1+G1-1; one 4-8KB line per partition
    ng = w.shape[1] // (G1 * P)
    r = w.astype(dt).reshape(KD, P, ng, G1, P)
    return np.ascontiguousarray(r.transpose(2, 1, 0, 3, 4))


def _pack_s2(w, dt):
    # [F, D] -> [NG2, 2, P, KF//2, G2, P]
    r = w.astype(dt).reshape(2, KF // 2, P, NG2, G2, P)
    return np.ascontiguousarray(r.transpose(3, 0, 2, 1, 4, 5))


def kernel(x, gate_w, w1, w3, w2, top_k):
    from concourse.bass_utils import run_bass_kernel_spmd

    x = np.asarray(x, np.float32)
    gate_w = np.asarray(gate_w, np.float32)
    w1 = np.asarray(w1, np.float32)
    w3 = np.asarray(w3, np.float32)
    w2 = np.asarray(w2, np.float32)
    k = int(top_k)

    x2d = x.reshape(T, D)
    sel, cw = _route(x2d, gate_w, k)

    # token lists per expert
    idx = [np.where((sel == e).any(axis=1))[0] for e in range(E)]
    wgt = []
    for e in range(E):
        m = sel[idx[e]] == e
        wgt.append(cw[idx[e]][m].astype(np.float32))
    counts = np.array([len(i) for i in idx])
    maxc = int(counts.max())
    C = max(128, -(-maxc // 16) * 16)
    n_chunks = 1
    if C > 512:  # capacity overflow: run multiple passes of 512
        C = 512
        n_chunks = -(-maxc // C)

    key = (C, W3_NFP8, W2_FP8)
    if key not in _cache:
        _cache[key] = _build(C, W3_NFP8, W2_FP8)
    nc = _cache[key]

    bf16 = _np_bf16()
    f8 = _np_f8()
    dt2 = f8 if W2_FP8 else bf16

    wpacked = []
    for e in range(E):
        # per-column scale for w3 (all columns, so w2pre rows stay at a
        # uniform magnitude); only the fp8 columns actually need it, but
        # scaling the bf16 columns too is free and exact to fold
        s3 = np.abs(w3[e]).max(axis=0) / F8MAX
        w3e = w3[e] / s3[None, :]
        # w2 rows absorb s3; its own per-column scale s2 is applied on host
        w2pre = w2[e] * s3[:, None]
        if W2_FP8:
            s2 = np.abs(w2pre).max(axis=0) / F8MAX
            w2pre = w2pre / s2[None, :]
        else:
            s2 = np.ones(D, np.float32)

        nf = W3_NFP8 * G1 * P
        maps = {"w1p": _pack_s1(w1[e], bf16), "w2p": _pack_s2(w2pre, dt2)}
        if W3_NFP8:
            maps["w3f"] = _pack_s1(w3e[:, :nf], f8)
        if W3_NFP8 < NG1:
            maps["w3b"] = _pack_s1(w3e[:, nf:], bf16)
        wpacked.append((maps, s2))

    out = np.zeros((T, D), np.float32)
    for chunk in range(n_chunks):
        in_maps = []
        for e in range(E):
            ide = idx[e][chunk * C:(chunk + 1) * C]
            xTe = np.zeros((D, C), bf16)
            xTe[:, :len(ide)] = x2d[ide].T.astype(bf16)
            m = dict(wpacked[e][0])
            m["xT"] = np.ascontiguousarray(
                xTe.reshape(KD, P, C).transpose(1, 0, 2))
            in_maps.append(m)
        res = run_bass_kernel_spmd(nc, in_maps, core_ids=list(range(E)))
        global last_results
        last_results = res
        for e in range(E):
            ide = idx[e][chunk * C:(chunk + 1) * C]
            if len(ide) == 0:
                continue
            we = wgt[e][chunk * C:(chunk + 1) * C]
            s2 = wpacked[e][1]
            # outT [NG2, P, G2, C] -> [D, C] with d = g*G2*P + m*P + p
            oTe = res.results[e]["outT"].transpose(0, 2, 1, 3).reshape(D, C)
            out[ide] += we[:, None] * (oTe[:, :len(ide)].T * s2[None, :])

    return out.reshape(B, S, D)
